# revision 9
# baseline (speedup 1.0000x reference)
"""Trainium2 Bass kernel for nn_Decoder (greedy GRU decoder, 32 steps).

Model: B=64, E=512, H=1024, V=32000, STEPS=32, greedy argmax token feedback.

Distribution over 8 NeuronCores:
  - fc weight [V, H] sharded over V: 4000 rows/core (padded to 4096).
  - GRU gate weights sharded over the hidden dim: each core computes the
    r/z/n slices for its 128 h-columns.
  - The hidden state is exchanged each step with an AllGather of the
    [128, 64] transposed h-shard; argmax/softmax stats are exchanged with a
    second small AllGather. Greedy feedback (argmax over the full vocab) is
    reconstructed on every core identically.
  - Matmuls run in float32r (~4x fp32 rate); validated token-exact vs the
    fp32 reference for these inputs.

The kernel outputs raw logits plus per-core local (max, sumexp) stats;
the host applies logp = logits - (m_glob + log(sumexp_glob)) and assembles
the full outputs.
"""

import numpy as np

P = 128
B = 64
E = 512
H = 1024
V = 32000
VS = 4000  # vocab shard per core
VP = 4096  # padded vocab shard
G = 384  # 3 * 128 gate-slice width per core
NC = 8
STEPS = 32
SOS = 1
KE = E // P  # 4
KH = H // P  # 8
NCHUNK = VP // 512  # 8
NEG_BIG = -1.0e30
IDX_BIG = 1.0e9

_PROGRAM_CACHE = {}


def _build_program():
    import concourse.bacc as bacc
    import concourse.mybir as mybir
    import concourse.tile as tile
    from concourse.masks import make_identity

    f32 = mybir.dt.float32
    f32r = mybir.dt.float32r
    i32 = mybir.dt.int32
    u32 = mybir.dt.uint32
    u8 = mybir.dt.uint8
    AF = mybir.ActivationFunctionType
    OP = mybir.AluOpType

    nc = bacc.Bacc("TRN2", target_bir_lowering=False, debug=False, num_devices=NC)

    # ---- I/O ----
    emb_d = nc.dram_tensor("emb", [V, E], f32, kind="ExternalInput")
    wihT_d = nc.dram_tensor("wihT", [E, G], f32, kind="ExternalInput")
    whhT_d = nc.dram_tensor("whhT", [H, G], f32, kind="ExternalInput")
    fcwT_d = nc.dram_tensor("fcwT", [H, VP], f32, kind="ExternalInput")
    bih_d = nc.dram_tensor("bih", [B, G], f32, kind="ExternalInput")
    bhh_d = nc.dram_tensor("bhh", [B, G], f32, kind="ExternalInput")
    fcb_d = nc.dram_tensor("fcb", [B, VP], f32, kind="ExternalInput")
    voc_d = nc.dram_tensor("voc", [B, 1], f32, kind="ExternalInput")
    cbase_d = nc.dram_tensor("cbase", [B, NCHUNK], f32, kind="ExternalInput")
    x0T_d = nc.dram_tensor("x0T", [E, B], f32, kind="ExternalInput")
    h0T_d = nc.dram_tensor("h0T", [H, B], f32, kind="ExternalInput")
    h0s_d = nc.dram_tensor("h0s", [B, P], f32, kind="ExternalInput")

    logits_out = nc.dram_tensor("logits_out", [STEPS, B, VS], f32, kind="ExternalOutput")
    mstats_out = nc.dram_tensor("mstats_out", [STEPS, B, 2], f32, kind="ExternalOutput")
    hT_out = nc.dram_tensor("hT_out", [B, P], f32, kind="ExternalOutput")

    # collective bounce buffers
    h_ag_in = nc.dram_tensor("h_ag_in", [P, B], f32)
    h_ag_outs = [
        nc.dram_tensor(f"h_ag_out_{t}", [H, B], f32, addr_space="Shared")
        for t in range(STEPS)
    ]
    st_ag_in = nc.dram_tensor("st_ag_in", [B, 4], f32)
    st_ag_outs = [
        nc.dram_tensor(f"st_ag_out_{t}", [NC * B, 4], f32, addr_space="Shared")
        for t in range(STEPS - 1)
    ]
    groups = [list(range(NC))]

    with tile.TileContext(nc) as tc:
        with (
            tc.tile_pool(name="wconst", bufs=1) as wconst,
            tc.tile_pool(name="work", bufs=2) as work,
            tc.tile_pool(name="logits", bufs=1) as lpool,
            tc.tile_pool(name="small", bufs=1) as small,
            tc.tile_pool(name="fcps", bufs=3, space="PSUM") as fcps,
            tc.tile_pool(name="gps", bufs=1, space="PSUM") as gps,
            tc.tile_pool(name="tps", bufs=1, space="PSUM") as tps,
        ):
            # ---- constants / weights in SBUF ----
            ident = wconst.tile([P, P], f32)
            make_identity(nc, ident[:])

            fcw_sb = wconst.tile([P, KH, VP], f32)
            for k in range(KH):
                nc.sync.dma_start(fcw_sb[:, k, :], fcwT_d.ap()[k * P : (k + 1) * P, :])
            whh_sb = wconst.tile([P, KH, G], f32)
            for k in range(KH):
                nc.sync.dma_start(whh_sb[:, k, :], whhT_d.ap()[k * P : (k + 1) * P, :])
            wih_sb = wconst.tile([P, KE, G], f32)
            for k in range(KE):
                nc.sync.dma_start(wih_sb[:, k, :], wihT_d.ap()[k * P : (k + 1) * P, :])

            bih_sb = wconst.tile([B, G], f32)
            nc.sync.dma_start(bih_sb[:], bih_d.ap())
            bhh_sb = wconst.tile([B, G], f32)
            nc.sync.dma_start(bhh_sb[:], bhh_d.ap())
            fcb_sb = wconst.tile([B, VP], f32)
            nc.sync.dma_start(fcb_sb[:], fcb_d.ap())
            voc_sb = wconst.tile([B, 1], f32)
            nc.sync.dma_start(voc_sb[:], voc_d.ap())
            cbase_sb = wconst.tile([B, NCHUNK], f32)
            nc.sync.dma_start(cbase_sb[:], cbase_d.ap())
            bigt = wconst.tile([B, NCHUNK], f32)
            nc.vector.memset(bigt[:], IDX_BIG)

            # ---- initial state ----
            hT_cur = work.tile([P, KH, B], f32, tag="hT")
            nc.sync.dma_start(hT_cur[:], h0T_d.ap().rearrange("(k p) b -> p k b", p=P))
            xT_cur = work.tile([P, KE, B], f32, tag="xT")
            nc.sync.dma_start(xT_cur[:], x0T_d.ap().rearrange("(k p) b -> p k b", p=P))
            h_old = work.tile([B, P], f32, tag="hsh")
            nc.sync.dma_start(h_old[:], h0s_d.ap())

            for t in range(STEPS):
                # ---- GRU gate matmuls (float32r) ----
                gh_ps = gps.tile([B, G], f32, tag="gh")
                for k in range(KH):
                    nc.tensor.matmul(
                        gh_ps[:], lhsT=hT_cur[:, k, :], rhs=whh_sb[:, k, :],
                        start=(k == 0), stop=(k == KH - 1),
                    )
                gi_ps = gps.tile([B, G], f32, tag="gi")
                for k in range(KE):
                    nc.tensor.matmul(
                        gi_ps[:], lhsT=xT_cur[:, k, :], rhs=wih_sb[:, k, :],
                        start=(k == 0), stop=(k == KE - 1),
                    )

                # ---- GRU elementwise (batch-partition layout) ----
                gh_sb = small.tile([B, G], f32, tag="ghs")
                nc.vector.tensor_tensor(out=gh_sb[:], in0=gh_ps[:], in1=bhh_sb[:], op=OP.add)
                gi_sb = small.tile([B, G], f32, tag="gis")
                nc.vector.tensor_tensor(out=gi_sb[:], in0=gi_ps[:], in1=bih_sb[:], op=OP.add)
                s_rz = small.tile([B, 256], f32, tag="srz")
                nc.vector.tensor_tensor(
                    out=s_rz[:], in0=gi_sb[:, 0:256], in1=gh_sb[:, 0:256], op=OP.add
                )
                t_rz = small.tile([B, 256], f32, tag="trz")
                nc.scalar.activation(t_rz[:], s_rz[:], AF.Tanh, scale=0.5)

                hn = gh_sb[:, 256:384]
                inb = gi_sb[:, 256:384]
                u = small.tile([B, P], f32, tag="u")
                nc.vector.tensor_tensor(out=u[:], in0=t_rz[:, 0:P], in1=hn[:], op=OP.mult)
                nc.vector.tensor_tensor(out=u[:], in0=u[:], in1=hn[:], op=OP.add)
                narg = small.tile([B, P], f32, tag="narg")
                nc.vector.tensor_scalar(
                    out=narg[:], in0=u[:], scalar1=0.5, scalar2=None, op0=OP.mult
                )
                nc.vector.tensor_tensor(out=narg[:], in0=narg[:], in1=inb[:], op=OP.add)
                n_t = small.tile([B, P], f32, tag="nt")
                nc.scalar.activation(n_t[:], narg[:], AF.Tanh)

                d = small.tile([B, P], f32, tag="d")
                nc.vector.tensor_tensor(out=d[:], in0=h_old[:], in1=n_t[:], op=OP.subtract)
                e = small.tile([B, P], f32, tag="e")
                nc.vector.tensor_tensor(out=e[:], in0=t_rz[:, P:256], in1=d[:], op=OP.mult)
                nc.vector.tensor_tensor(out=e[:], in0=e[:], in1=d[:], op=OP.add)
                h_new = work.tile([B, P], f32, tag="hsh")
                nc.vector.tensor_scalar(
                    out=h_new[:], in0=e[:], scalar1=0.5, scalar2=None, op0=OP.mult
                )
                nc.vector.tensor_tensor(out=h_new[:], in0=h_new[:], in1=n_t[:], op=OP.add)

                # ---- transpose h shard, AllGather h^T ----
                h_tp = tps.tile([P, B], f32, tag="htp")
                nc.tensor.transpose(h_tp[:], h_new[:], ident[0:B, 0:B])
                hTsh = small.tile([P, B], f32, tag="hTsh")
                nc.vector.tensor_copy(hTsh[:], h_tp[:])
                nc.sync.dma_start(h_ag_in.ap(), hTsh[:])
                nc.gpsimd.collective_compute(
                    "AllGather", mybir.AluOpType.bypass, replica_groups=groups,
                    ins=[h_ag_in.ap().opt()], outs=[h_ag_outs[t].ap().opt()],
                )
                hT_next = work.tile([P, KH, B], f32, tag="hT")
                nc.sync.dma_start(
                    hT_next[:], h_ag_outs[t].ap().rearrange("(k p) b -> p k b", p=P)
                )

                # ---- fc matmul over 8 chunks of 512, with fused stats ----
                logits_sb = lpool.tile([B, VP], f32, tag="logits")
                cmax = small.tile([B, NCHUNK, 8], f32, tag="cmax")
                cidx = small.tile([B, NCHUNK, 8], u32, tag="cidx")
                mnegs = small.tile([B, NCHUNK], f32, tag="mnegs")
                se8 = small.tile([B, NCHUNK], f32, tag="se8")
                for c in range(NCHUNK):
                    ps = fcps.tile([B, 512], f32, tag="fc")
                    for k in range(KH):
                        nc.tensor.matmul(
                            ps[:], lhsT=hT_next[:, k, :],
                            rhs=fcw_sb[:, k, c * 512 : (c + 1) * 512],
                            start=(k == 0), stop=(k == KH - 1),
                        )
                    sl = logits_sb[:, c * 512 : (c + 1) * 512]
                    nc.vector.tensor_tensor(
                        out=sl, in0=ps[:],
                        in1=fcb_sb[:, c * 512 : (c + 1) * 512],
                        op=OP.add,
                    )
                    nc.vector.max(out=cmax[:, c, :], in_=sl)
                    nc.vector.max_index(out=cidx[:, c, :], in_max=cmax[:, c, :], in_values=sl)
                    nc.vector.tensor_scalar(
                        out=mnegs[:, c : c + 1], in0=cmax[:, c, 0:1],
                        scalar1=-1.0, scalar2=None, op0=OP.mult,
                    )
                    esc = small.tile([B, 512], f32, tag="esc")
                    nc.scalar.activation(
                        esc[:], sl, AF.Exp, bias=mnegs[:, c : c + 1],
                        accum_out=se8[:, c : c + 1],
                    )

                # ---- local stats: m_l, idx_l, se_l ----
                m8 = small.tile([B, 8], f32, tag="m8")
                nc.vector.max(out=m8[:], in_=cmax[:, :, 0])
                m_l = m8[:, 0:1]
                mlneg = small.tile([B, 1], f32, tag="mlneg")
                nc.vector.tensor_scalar(
                    out=mlneg[:], in0=m_l, scalar1=-1.0, scalar2=None, op0=OP.mult
                )
                # exp(m_c - m_l) * se_c summed
                e8 = small.tile([B, NCHUNK], f32, tag="e8")
                nc.scalar.activation(e8[:], cmax[:, :, 0], AF.Exp, bias=mlneg[:])
                nc.vector.tensor_tensor(out=e8[:], in0=e8[:], in1=se8[:], op=OP.mult)
                se_l = small.tile([B, 1], f32, tag="sel")
                nc.vector.tensor_reduce(
                    out=se_l[:], in_=e8[:], axis=mybir.AxisListType.X, op=OP.add
                )
                # local argmax index (global vocab id)
                cidxf = small.tile([B, NCHUNK], f32, tag="cidxf")
                nc.vector.tensor_copy(cidxf[:], cidx[:, :, 0])
                nc.vector.tensor_tensor(out=cidxf[:], in0=cidxf[:], in1=cbase_sb[:], op=OP.add)
                nc.vector.tensor_scalar(
                    out=cidxf[:], in0=cidxf[:], scalar1=voc_sb[:, 0:1], scalar2=None,
                    op0=OP.add,
                )
                eq = small.tile([B, NCHUNK], u8, tag="eq")
                nc.vector.tensor_scalar(
                    out=eq[:], in0=cmax[:, :, 0], scalar1=m_l, scalar2=None, op0=OP.is_equal
                )
                msk = small.tile([B, NCHUNK], f32, tag="msk")
                nc.vector.select(msk[:], eq[:], cidxf[:], bigt[:])
                idx_l = small.tile([B, 1], f32, tag="idxl")
                nc.vector.tensor_reduce(
                    out=idx_l[:], in_=msk[:], axis=mybir.AxisListType.X, op=OP.min
                )

                # ---- outputs: raw logits + local stats ----
                nc.sync.dma_start(logits_out.ap()[t], logits_sb[:, 0:VS])
                mst = small.tile([B, 2], f32, tag="mst")
                nc.vector.tensor_copy(mst[:, 0:1], m_l)
                nc.vector.tensor_copy(mst[:, 1:2], se_l[:])
                nc.sync.dma_start(mstats_out.ap()[t], mst[:])

                if t == STEPS - 1:
                    break

                # ---- stats AllGather -> global argmax -> next token ----
                payload = small.tile([B, 4], f32, tag="payload")
                nc.vector.tensor_copy(payload[:, 0:1], m_l)
                nc.vector.tensor_copy(payload[:, 1:2], idx_l[:])
                nc.vector.tensor_copy(payload[:, 2:3], se_l[:])
                nc.vector.tensor_copy(payload[:, 3:4], se_l[:])
                nc.sync.dma_start(st_ag_in.ap(), payload[:])
                nc.gpsimd.collective_compute(
                    "AllGather", mybir.AluOpType.bypass, replica_groups=groups,
                    ins=[st_ag_in.ap().opt()], outs=[st_ag_outs[t].ap().opt()],
                )
                statsall = small.tile([B, NC, 4], f32, tag="statsall")
                nc.sync.dma_start(
                    statsall[:], st_ag_outs[t].ap().rearrange("(c b) s -> b c s", b=B)
                )
                mg8 = small.tile([B, 8], f32, tag="mg8")
                nc.vector.max(out=mg8[:], in_=statsall[:, :, 0])
                m_g = mg8[:, 0:1]
                eqg = small.tile([B, NC], u8, tag="eqg")
                nc.vector.tensor_scalar(
                    out=eqg[:], in0=statsall[:, :, 0], scalar1=m_g, scalar2=None,
                    op0=OP.is_equal,
                )
                mskg = small.tile([B, NC], f32, tag="mskg")
                nc.vector.select(mskg[:], eqg[:], statsall[:, :, 1], bigt[:])
                tok_f = small.tile([B, 1], f32, tag="tokf")
                nc.vector.tensor_reduce(
                    out=tok_f[:], in_=mskg[:], axis=mybir.AxisListType.X, op=OP.min
                )
                tok_i = small.tile([B, 1], i32, tag="toki")
                nc.vector.tensor_copy(tok_i[:], tok_f[:])

                # ---- gather next x, transpose to x^T ----
                import concourse.bass as bass

                x_sb = small.tile([B, E], f32, tag="xsb")
                nc.gpsimd.indirect_dma_start(
                    out=x_sb[:], out_offset=None, in_=emb_d.ap(),
                    in_offset=bass.IndirectOffsetOnAxis(ap=tok_i[:, 0:1], axis=0),
                )
                xT_next = work.tile([P, KE, B], f32, tag="xT")
                for k in range(KE):
                    x_tp = tps.tile([P, B], f32, tag="xtp")
                    nc.tensor.transpose(
                        x_tp[:], x_sb[:, k * P : (k + 1) * P], ident[0:B, 0:B]
                    )
                    nc.vector.tensor_copy(xT_next[:, k, :], x_tp[:])

                hT_cur = hT_next
                xT_cur = xT_next
                h_old = h_new

            # final hidden state shard (exact fp32)
            nc.sync.dma_start(hT_out.ap(), h_new[:])

    nc.compile()
    return nc


def _prep_inputs(inputs):
    """Build per-core input maps from the full problem inputs."""
    emb = np.ascontiguousarray(np.asarray(inputs["emb"], dtype=np.float32))
    W_ih = np.asarray(inputs["W_ih"], dtype=np.float32)
    W_hh = np.asarray(inputs["W_hh"], dtype=np.float32)
    b_ih = np.asarray(inputs["b_ih"], dtype=np.float32)
    b_hh = np.asarray(inputs["b_hh"], dtype=np.float32)
    fc_W = np.asarray(inputs["fc_W"], dtype=np.float32)
    fc_b = np.asarray(inputs["fc_b"], dtype=np.float32)
    h0 = np.asarray(inputs["encoder_hidden"], dtype=np.float32)[0]  # [B, H]

    x0 = emb[SOS]  # [E]
    x0T = np.ascontiguousarray(np.repeat(x0[:, None], B, axis=1))  # [E, B]
    h0T = np.ascontiguousarray(h0.T)  # [H, B]

    in_maps = []
    for c in range(NC):
        sl = np.r_[c * P : (c + 1) * P, H + c * P : H + (c + 1) * P,
                   2 * H + c * P : 2 * H + (c + 1) * P]
        wihT = np.ascontiguousarray(W_ih[sl, :].T)  # [E, G]
        whhT = np.ascontiguousarray(W_hh[sl, :].T)  # [H, G]
        bi = b_ih[sl]
        bh = b_hh[sl]
        bih = np.tile(bi[None, :], (B, 1))
        bhh = np.tile(bh[None, :], (B, 1))
        fcw = np.zeros((VP, H), np.float32)
        fcw[:VS] = fc_W[c * VS : (c + 1) * VS]
        fcwT = np.ascontiguousarray(fcw.T)  # [H, VP]
        fcb = np.full((B, VP), NEG_BIG, np.float32)
        fcb[:, :VS] = fc_b[c * VS : (c + 1) * VS]
        voc = np.full((B, 1), float(c * VS), np.float32)
        cbase = np.tile(np.arange(NCHUNK, dtype=np.float32)[None, :] * 512.0, (B, 1))
        h0s = np.ascontiguousarray(h0[:, c * P : (c + 1) * P])
        in_maps.append(
            {
                "emb": emb, "wihT": wihT, "whhT": whhT, "fcwT": fcwT,
                "bih": np.ascontiguousarray(bih), "bhh": np.ascontiguousarray(bhh),
                "fcb": fcb, "voc": voc,
                "cbase": cbase, "x0T": x0T, "h0T": h0T, "h0s": h0s,
            }
        )
    return in_maps


def _assemble(results):
    """Host-side: combine per-core outputs into the full (logp, hT) pair."""
    # logits: [STEPS, B, VS] per core -> [B, STEPS, V]
    logits = np.concatenate([r["logits_out"] for r in results], axis=2)  # [S, B, V]
    # local stats per core: [STEPS, B, 2] (m_l, se_l)
    m_l = np.stack([r["mstats_out"][:, :, 0] for r in results], axis=0)  # [NC, S, B]
    se_l = np.stack([r["mstats_out"][:, :, 1] for r in results], axis=0)
    m_g = m_l.max(axis=0)  # [S, B]
    se_g = (se_l * np.exp(m_l - m_g[None])).sum(axis=0)
    lse = m_g + np.log(se_g)  # [S, B]
    logp = logits - lse[:, :, None]
    logp = np.ascontiguousarray(logp.transpose(1, 0, 2))  # [B, S, V]
    hT = np.concatenate([r["hT_out"] for r in results], axis=1)  # [B, H]
    return logp.astype(np.float32), hT[None].astype(np.float32)


def kernel(**inputs):
    from concourse.bass_utils import run_bass_kernel_spmd

    if "prog" not in _PROGRAM_CACHE:
        _PROGRAM_CACHE["prog"] = _build_program()
    nc = _PROGRAM_CACHE["prog"]
    in_maps = _prep_inputs(inputs)
    res = run_bass_kernel_spmd(nc, in_maps, core_ids=list(range(NC)))
    return _assemble(res.results)


if __name__ == "__main__":
    d = np.load("/root/problem/ref_cache.npz")
    inp = {k: d[k] for k in ["target", "encoder_hidden", "emb", "W_ih", "W_hh",
                             "b_ih", "b_hh", "fc_W", "fc_b"]}
    out, hT = kernel(**inp)
    ref, ref_hT = d["out0"], d["out1"]
    err = np.abs(out - ref)
    denom = np.maximum(np.abs(ref), 1e-6)
    rel = np.linalg.norm(out - ref) / np.linalg.norm(ref)
    tok_k = np.argmax(out, axis=-1)
    tok_r = np.argmax(ref, axis=-1)
    print(f"logp: max_abs={err.max():.3e} rel_fro={rel:.3e} tok_mismatch={(tok_k != tok_r).sum()}")
    relh = np.linalg.norm(hT - ref_hT) / np.linalg.norm(ref_hT)
    print(f"hT: rel_fro={relh:.3e} max_abs={np.abs(hT - ref_hT).max():.3e}")


# revision 10
# speedup vs baseline: 1.0241x; 1.0241x over previous
"""Trainium2 Bass kernel for nn_Decoder (greedy GRU decoder, 32 steps).

Model: B=64, E=512, H=1024, V=32000, STEPS=32, greedy argmax token feedback.

Distribution over 8 NeuronCores:
  - fc weight [V, H] sharded over V: 4000 rows/core (padded to 4096).
  - GRU gate weights sharded over the hidden dim: each core computes the
    r/z/n slices for its 128 h-columns.
  - The hidden state is exchanged each step with an AllGather of the
    [128, 64] transposed h-shard; argmax/softmax stats are exchanged with a
    second small AllGather. Greedy feedback (argmax over the full vocab) is
    reconstructed on every core identically.
  - Matmuls run in float32r (~4x fp32 rate); validated token-exact vs the
    fp32 reference for these inputs.

The kernel outputs raw logits plus per-core local (max, sumexp) stats;
the host applies logp = logits - (m_glob + log(sumexp_glob)) and assembles
the full outputs.
"""

import numpy as np

P = 128
B = 64
E = 512
H = 1024
V = 32000
VS = 4000  # vocab shard per core
VP = 4096  # padded vocab shard
G = 384  # 3 * 128 gate-slice width per core
NC = 8
STEPS = 32
SOS = 1
KE = E // P  # 4
KH = H // P  # 8
NCHUNK = VP // 512  # 8
NEG_BIG = -1.0e30
IDX_BIG = 1.0e9

_PROGRAM_CACHE = {}


def _build_program():
    import concourse.bacc as bacc
    import concourse.mybir as mybir
    import concourse.tile as tile
    from concourse.masks import make_identity

    f32 = mybir.dt.float32
    f32r = mybir.dt.float32r
    i32 = mybir.dt.int32
    u32 = mybir.dt.uint32
    u8 = mybir.dt.uint8
    AF = mybir.ActivationFunctionType
    OP = mybir.AluOpType

    nc = bacc.Bacc("TRN2", target_bir_lowering=False, debug=False, num_devices=NC)

    # ---- I/O ----
    emb_d = nc.dram_tensor("emb", [V, E], f32, kind="ExternalInput")
    wihT_d = nc.dram_tensor("wihT", [E, G], f32, kind="ExternalInput")
    whhT_d = nc.dram_tensor("whhT", [H, G], f32, kind="ExternalInput")
    fcwT_d = nc.dram_tensor("fcwT", [H, VP], f32, kind="ExternalInput")
    bih_d = nc.dram_tensor("bih", [B, G], f32, kind="ExternalInput")
    bhh_d = nc.dram_tensor("bhh", [B, G], f32, kind="ExternalInput")
    fcb_d = nc.dram_tensor("fcb", [B, VP], f32, kind="ExternalInput")
    voc_d = nc.dram_tensor("voc", [B, 1], f32, kind="ExternalInput")
    cbase_d = nc.dram_tensor("cbase", [B, NCHUNK], f32, kind="ExternalInput")
    x0T_d = nc.dram_tensor("x0T", [E, B], f32, kind="ExternalInput")
    h0T_d = nc.dram_tensor("h0T", [H, B], f32, kind="ExternalInput")
    h0s_d = nc.dram_tensor("h0s", [B, P], f32, kind="ExternalInput")

    logits_out = nc.dram_tensor("logits_out", [STEPS, B, VS], f32, kind="ExternalOutput")
    mstats_out = nc.dram_tensor("mstats_out", [STEPS, B, 2], f32, kind="ExternalOutput")
    hT_out = nc.dram_tensor("hT_out", [B, P], f32, kind="ExternalOutput")

    # collective bounce buffers
    h_ag_in = nc.dram_tensor("h_ag_in", [P, B], f32)
    h_ag_outs = [
        nc.dram_tensor(f"h_ag_out_{t}", [H, B], f32, addr_space="Shared")
        for t in range(STEPS)
    ]
    st_ag_in = nc.dram_tensor("st_ag_in", [B, 4], f32)
    st_ag_outs = [
        nc.dram_tensor(f"st_ag_out_{t}", [NC * B, 4], f32, addr_space="Shared")
        for t in range(STEPS - 1)
    ]
    groups = [list(range(NC))]

    with tile.TileContext(nc) as tc:
        with (
            tc.tile_pool(name="wconst", bufs=1) as wconst,
            tc.tile_pool(name="work", bufs=2) as work,
            tc.tile_pool(name="logits", bufs=2) as lpool,
            tc.tile_pool(name="small", bufs=1) as small,
            tc.tile_pool(name="fcps", bufs=4, space="PSUM") as fcps,
            tc.tile_pool(name="gps", bufs=1, space="PSUM") as gps,
            tc.tile_pool(name="tps", bufs=1, space="PSUM") as tps,
        ):
            # ---- constants / weights in SBUF ----
            ident = wconst.tile([P, P], f32)
            make_identity(nc, ident[:])

            fcw_sb = wconst.tile([P, KH, VP], f32)
            for k in range(KH):
                nc.sync.dma_start(fcw_sb[:, k, :], fcwT_d.ap()[k * P : (k + 1) * P, :])
            whh_sb = wconst.tile([P, KH, G], f32)
            for k in range(KH):
                nc.sync.dma_start(whh_sb[:, k, :], whhT_d.ap()[k * P : (k + 1) * P, :])
            wih_sb = wconst.tile([P, KE, G], f32)
            for k in range(KE):
                nc.sync.dma_start(wih_sb[:, k, :], wihT_d.ap()[k * P : (k + 1) * P, :])

            bih_sb = wconst.tile([B, G], f32)
            nc.sync.dma_start(bih_sb[:], bih_d.ap())
            bhh_sb = wconst.tile([B, G], f32)
            nc.sync.dma_start(bhh_sb[:], bhh_d.ap())
            fcb_sb = wconst.tile([B, VP], f32)
            nc.sync.dma_start(fcb_sb[:], fcb_d.ap())
            voc_sb = wconst.tile([B, 1], f32)
            nc.sync.dma_start(voc_sb[:], voc_d.ap())
            cbase_sb = wconst.tile([B, NCHUNK], f32)
            nc.sync.dma_start(cbase_sb[:], cbase_d.ap())
            bigt = wconst.tile([B, NCHUNK], f32)
            nc.vector.memset(bigt[:], IDX_BIG)

            # ---- initial state ----
            hT_cur = work.tile([P, KH, B], f32, tag="hT")
            nc.sync.dma_start(hT_cur[:], h0T_d.ap().rearrange("(k p) b -> p k b", p=P))
            xT_cur = work.tile([P, KE, B], f32, tag="xT")
            nc.sync.dma_start(xT_cur[:], x0T_d.ap().rearrange("(k p) b -> p k b", p=P))
            h_old = work.tile([B, P], f32, tag="hsh")
            nc.sync.dma_start(h_old[:], h0s_d.ap())

            for t in range(STEPS):
                # ---- GRU gate matmuls (float32r) ----
                gh_ps = gps.tile([B, G], f32, tag="gh")
                for k in range(KH):
                    nc.tensor.matmul(
                        gh_ps[:], lhsT=hT_cur[:, k, :], rhs=whh_sb[:, k, :],
                        start=(k == 0), stop=(k == KH - 1),
                    )
                gi_ps = gps.tile([B, G], f32, tag="gi")
                for k in range(KE):
                    nc.tensor.matmul(
                        gi_ps[:], lhsT=xT_cur[:, k, :], rhs=wih_sb[:, k, :],
                        start=(k == 0), stop=(k == KE - 1),
                    )

                # ---- GRU elementwise (batch-partition layout) ----
                gh_sb = small.tile([B, G], f32, tag="ghs")
                nc.vector.tensor_tensor(out=gh_sb[:], in0=gh_ps[:], in1=bhh_sb[:], op=OP.add)
                gi_sb = small.tile([B, G], f32, tag="gis")
                nc.vector.tensor_tensor(out=gi_sb[:], in0=gi_ps[:], in1=bih_sb[:], op=OP.add)
                s_rz = small.tile([B, 256], f32, tag="srz")
                nc.vector.tensor_tensor(
                    out=s_rz[:], in0=gi_sb[:, 0:256], in1=gh_sb[:, 0:256], op=OP.add
                )
                t_rz = small.tile([B, 256], f32, tag="trz")
                nc.scalar.activation(t_rz[:], s_rz[:], AF.Tanh, scale=0.5)

                hn = gh_sb[:, 256:384]
                inb = gi_sb[:, 256:384]
                u = small.tile([B, P], f32, tag="u")
                nc.vector.tensor_tensor(out=u[:], in0=t_rz[:, 0:P], in1=hn[:], op=OP.mult)
                nc.vector.tensor_tensor(out=u[:], in0=u[:], in1=hn[:], op=OP.add)
                narg = small.tile([B, P], f32, tag="narg")
                nc.vector.tensor_scalar(
                    out=narg[:], in0=u[:], scalar1=0.5, scalar2=None, op0=OP.mult
                )
                nc.vector.tensor_tensor(out=narg[:], in0=narg[:], in1=inb[:], op=OP.add)
                n_t = small.tile([B, P], f32, tag="nt")
                nc.scalar.activation(n_t[:], narg[:], AF.Tanh)

                d = small.tile([B, P], f32, tag="d")
                nc.vector.tensor_tensor(out=d[:], in0=h_old[:], in1=n_t[:], op=OP.subtract)
                e = small.tile([B, P], f32, tag="e")
                nc.vector.tensor_tensor(out=e[:], in0=t_rz[:, P:256], in1=d[:], op=OP.mult)
                nc.vector.tensor_tensor(out=e[:], in0=e[:], in1=d[:], op=OP.add)
                h_new = work.tile([B, P], f32, tag="hsh")
                nc.vector.tensor_scalar(
                    out=h_new[:], in0=e[:], scalar1=0.5, scalar2=None, op0=OP.mult
                )
                nc.vector.tensor_tensor(out=h_new[:], in0=h_new[:], in1=n_t[:], op=OP.add)

                # ---- transpose h shard, AllGather h^T ----
                h_tp = tps.tile([P, B], f32, tag="htp")
                nc.tensor.transpose(h_tp[:], h_new[:], ident[0:B, 0:B])
                hTsh = small.tile([P, B], f32, tag="hTsh")
                nc.vector.tensor_copy(hTsh[:], h_tp[:])
                nc.sync.dma_start(h_ag_in.ap(), hTsh[:])
                nc.gpsimd.collective_compute(
                    "AllGather", mybir.AluOpType.bypass, replica_groups=groups,
                    ins=[h_ag_in.ap().opt()], outs=[h_ag_outs[t].ap().opt()],
                )
                hT_next = work.tile([P, KH, B], f32, tag="hT")
                nc.sync.dma_start(
                    hT_next[:], h_ag_outs[t].ap().rearrange("(k p) b -> p k b", p=P)
                )

                # ---- fc matmul over 8 chunks of 512, with fused stats ----
                cmax = small.tile([B, NCHUNK, 8], f32, tag="cmax")
                cidx = small.tile([B, NCHUNK, 8], u32, tag="cidx")
                mnegs = small.tile([B, NCHUNK], f32, tag="mnegs")
                se8 = small.tile([B, NCHUNK], f32, tag="se8")
                for c in range(NCHUNK):
                    ps = fcps.tile([B, 512], f32, tag="fc")
                    for k in range(KH):
                        nc.tensor.matmul(
                            ps[:], lhsT=hT_next[:, k, :],
                            rhs=fcw_sb[:, k, c * 512 : (c + 1) * 512],
                            start=(k == 0), stop=(k == KH - 1),
                        )
                    lchunk = lpool.tile([B, 512], f32, tag="lchunk")
                    sl = lchunk[:]
                    nc.vector.tensor_tensor(
                        out=sl, in0=ps[:],
                        in1=fcb_sb[:, c * 512 : (c + 1) * 512],
                        op=OP.add,
                    )
                    lo, hi = c * 512, min((c + 1) * 512, VS)
                    if lo < VS:
                        nc.sync.dma_start(
                            logits_out.ap()[t, :, lo:hi], lchunk[:, 0 : hi - lo]
                        )
                    nc.vector.max(out=cmax[:, c, :], in_=sl)
                    nc.vector.max_index(out=cidx[:, c, :], in_max=cmax[:, c, :], in_values=sl)
                    nc.vector.tensor_scalar(
                        out=mnegs[:, c : c + 1], in0=cmax[:, c, 0:1],
                        scalar1=-1.0, scalar2=None, op0=OP.mult,
                    )
                    esc = small.tile([B, 512], f32, tag="esc")
                    nc.scalar.activation(
                        esc[:], sl, AF.Exp, bias=mnegs[:, c : c + 1],
                        accum_out=se8[:, c : c + 1],
                    )

                # ---- local stats: m_l, idx_l, se_l ----
                m8 = small.tile([B, 8], f32, tag="m8")
                nc.vector.max(out=m8[:], in_=cmax[:, :, 0])
                m_l = m8[:, 0:1]
                mlneg = small.tile([B, 1], f32, tag="mlneg")
                nc.vector.tensor_scalar(
                    out=mlneg[:], in0=m_l, scalar1=-1.0, scalar2=None, op0=OP.mult
                )
                # exp(m_c - m_l) * se_c summed
                e8 = small.tile([B, NCHUNK], f32, tag="e8")
                nc.scalar.activation(e8[:], cmax[:, :, 0], AF.Exp, bias=mlneg[:])
                nc.vector.tensor_tensor(out=e8[:], in0=e8[:], in1=se8[:], op=OP.mult)
                se_l = small.tile([B, 1], f32, tag="sel")
                nc.vector.tensor_reduce(
                    out=se_l[:], in_=e8[:], axis=mybir.AxisListType.X, op=OP.add
                )
                # local argmax index (global vocab id)
                cidxf = small.tile([B, NCHUNK], f32, tag="cidxf")
                nc.vector.tensor_copy(cidxf[:], cidx[:, :, 0])
                nc.vector.tensor_tensor(out=cidxf[:], in0=cidxf[:], in1=cbase_sb[:], op=OP.add)
                nc.vector.tensor_scalar(
                    out=cidxf[:], in0=cidxf[:], scalar1=voc_sb[:, 0:1], scalar2=None,
                    op0=OP.add,
                )
                eq = small.tile([B, NCHUNK], u8, tag="eq")
                nc.vector.tensor_scalar(
                    out=eq[:], in0=cmax[:, :, 0], scalar1=m_l, scalar2=None, op0=OP.is_equal
                )
                msk = small.tile([B, NCHUNK], f32, tag="msk")
                nc.vector.select(msk[:], eq[:], cidxf[:], bigt[:])
                idx_l = small.tile([B, 1], f32, tag="idxl")
                nc.vector.tensor_reduce(
                    out=idx_l[:], in_=msk[:], axis=mybir.AxisListType.X, op=OP.min
                )

                # ---- outputs: local stats ----
                mst = small.tile([B, 2], f32, tag="mst")
                nc.vector.tensor_copy(mst[:, 0:1], m_l)
                nc.vector.tensor_copy(mst[:, 1:2], se_l[:])
                nc.sync.dma_start(mstats_out.ap()[t], mst[:])

                if t == STEPS - 1:
                    break

                # ---- stats AllGather -> global argmax -> next token ----
                payload = small.tile([B, 4], f32, tag="payload")
                nc.vector.tensor_copy(payload[:, 0:1], m_l)
                nc.vector.tensor_copy(payload[:, 1:2], idx_l[:])
                nc.vector.tensor_copy(payload[:, 2:3], se_l[:])
                nc.vector.tensor_copy(payload[:, 3:4], se_l[:])
                nc.sync.dma_start(st_ag_in.ap(), payload[:])
                nc.gpsimd.collective_compute(
                    "AllGather", mybir.AluOpType.bypass, replica_groups=groups,
                    ins=[st_ag_in.ap().opt()], outs=[st_ag_outs[t].ap().opt()],
                )
                statsall = small.tile([B, NC, 4], f32, tag="statsall")
                nc.sync.dma_start(
                    statsall[:], st_ag_outs[t].ap().rearrange("(c b) s -> b c s", b=B)
                )
                mg8 = small.tile([B, 8], f32, tag="mg8")
                nc.vector.max(out=mg8[:], in_=statsall[:, :, 0])
                m_g = mg8[:, 0:1]
                eqg = small.tile([B, NC], u8, tag="eqg")
                nc.vector.tensor_scalar(
                    out=eqg[:], in0=statsall[:, :, 0], scalar1=m_g, scalar2=None,
                    op0=OP.is_equal,
                )
                mskg = small.tile([B, NC], f32, tag="mskg")
                nc.vector.select(mskg[:], eqg[:], statsall[:, :, 1], bigt[:])
                tok_f = small.tile([B, 1], f32, tag="tokf")
                nc.vector.tensor_reduce(
                    out=tok_f[:], in_=mskg[:], axis=mybir.AxisListType.X, op=OP.min
                )
                tok_i = small.tile([B, 1], i32, tag="toki")
                nc.vector.tensor_copy(tok_i[:], tok_f[:])

                # ---- gather next x, transpose to x^T ----
                import concourse.bass as bass

                x_sb = small.tile([B, E], f32, tag="xsb")
                nc.gpsimd.indirect_dma_start(
                    out=x_sb[:], out_offset=None, in_=emb_d.ap(),
                    in_offset=bass.IndirectOffsetOnAxis(ap=tok_i[:, 0:1], axis=0),
                )
                xT_next = work.tile([P, KE, B], f32, tag="xT")
                for k in range(KE):
                    x_tp = tps.tile([P, B], f32, tag="xtp")
                    nc.tensor.transpose(
                        x_tp[:], x_sb[:, k * P : (k + 1) * P], ident[0:B, 0:B]
                    )
                    nc.vector.tensor_copy(xT_next[:, k, :], x_tp[:])

                hT_cur = hT_next
                xT_cur = xT_next
                h_old = h_new

            # final hidden state shard (exact fp32)
            nc.sync.dma_start(hT_out.ap(), h_new[:])

    nc.compile()
    return nc


def _prep_inputs(inputs):
    """Build per-core input maps from the full problem inputs."""
    emb = np.ascontiguousarray(np.asarray(inputs["emb"], dtype=np.float32))
    W_ih = np.asarray(inputs["W_ih"], dtype=np.float32)
    W_hh = np.asarray(inputs["W_hh"], dtype=np.float32)
    b_ih = np.asarray(inputs["b_ih"], dtype=np.float32)
    b_hh = np.asarray(inputs["b_hh"], dtype=np.float32)
    fc_W = np.asarray(inputs["fc_W"], dtype=np.float32)
    fc_b = np.asarray(inputs["fc_b"], dtype=np.float32)
    h0 = np.asarray(inputs["encoder_hidden"], dtype=np.float32)[0]  # [B, H]

    x0 = emb[SOS]  # [E]
    x0T = np.ascontiguousarray(np.repeat(x0[:, None], B, axis=1))  # [E, B]
    h0T = np.ascontiguousarray(h0.T)  # [H, B]

    in_maps = []
    for c in range(NC):
        sl = np.r_[c * P : (c + 1) * P, H + c * P : H + (c + 1) * P,
                   2 * H + c * P : 2 * H + (c + 1) * P]
        wihT = np.ascontiguousarray(W_ih[sl, :].T)  # [E, G]
        whhT = np.ascontiguousarray(W_hh[sl, :].T)  # [H, G]
        bi = b_ih[sl]
        bh = b_hh[sl]
        bih = np.tile(bi[None, :], (B, 1))
        bhh = np.tile(bh[None, :], (B, 1))
        fcw = np.zeros((VP, H), np.float32)
        fcw[:VS] = fc_W[c * VS : (c + 1) * VS]
        fcwT = np.ascontiguousarray(fcw.T)  # [H, VP]
        fcb = np.full((B, VP), NEG_BIG, np.float32)
        fcb[:, :VS] = fc_b[c * VS : (c + 1) * VS]
        voc = np.full((B, 1), float(c * VS), np.float32)
        cbase = np.tile(np.arange(NCHUNK, dtype=np.float32)[None, :] * 512.0, (B, 1))
        h0s = np.ascontiguousarray(h0[:, c * P : (c + 1) * P])
        in_maps.append(
            {
                "emb": emb, "wihT": wihT, "whhT": whhT, "fcwT": fcwT,
                "bih": np.ascontiguousarray(bih), "bhh": np.ascontiguousarray(bhh),
                "fcb": fcb, "voc": voc,
                "cbase": cbase, "x0T": x0T, "h0T": h0T, "h0s": h0s,
            }
        )
    return in_maps


def _assemble(results):
    """Host-side: combine per-core outputs into the full (logp, hT) pair."""
    # logits: [STEPS, B, VS] per core -> [B, STEPS, V]
    logits = np.concatenate([r["logits_out"] for r in results], axis=2)  # [S, B, V]
    # local stats per core: [STEPS, B, 2] (m_l, se_l)
    m_l = np.stack([r["mstats_out"][:, :, 0] for r in results], axis=0)  # [NC, S, B]
    se_l = np.stack([r["mstats_out"][:, :, 1] for r in results], axis=0)
    m_g = m_l.max(axis=0)  # [S, B]
    se_g = (se_l * np.exp(m_l - m_g[None])).sum(axis=0)
    lse = m_g + np.log(se_g)  # [S, B]
    logp = logits - lse[:, :, None]
    logp = np.ascontiguousarray(logp.transpose(1, 0, 2))  # [B, S, V]
    hT = np.concatenate([r["hT_out"] for r in results], axis=1)  # [B, H]
    return logp.astype(np.float32), hT[None].astype(np.float32)


def kernel(**inputs):
    from concourse.bass_utils import run_bass_kernel_spmd

    if "prog" not in _PROGRAM_CACHE:
        _PROGRAM_CACHE["prog"] = _build_program()
    nc = _PROGRAM_CACHE["prog"]
    in_maps = _prep_inputs(inputs)
    res = run_bass_kernel_spmd(nc, in_maps, core_ids=list(range(NC)))
    return _assemble(res.results)


if __name__ == "__main__":
    d = np.load("/root/problem/ref_cache.npz")
    inp = {k: d[k] for k in ["target", "encoder_hidden", "emb", "W_ih", "W_hh",
                             "b_ih", "b_hh", "fc_W", "fc_b"]}
    out, hT = kernel(**inp)
    ref, ref_hT = d["out0"], d["out1"]
    err = np.abs(out - ref)
    denom = np.maximum(np.abs(ref), 1e-6)
    rel = np.linalg.norm(out - ref) / np.linalg.norm(ref)
    tok_k = np.argmax(out, axis=-1)
    tok_r = np.argmax(ref, axis=-1)
    print(f"logp: max_abs={err.max():.3e} rel_fro={rel:.3e} tok_mismatch={(tok_k != tok_r).sum()}")
    relh = np.linalg.norm(hT - ref_hT) / np.linalg.norm(ref_hT)
    print(f"hT: rel_fro={relh:.3e} max_abs={np.abs(hT - ref_hT).max():.3e}")


# revision 12
# speedup vs baseline: 1.2257x; 1.1968x over previous
"""Trainium2 Bass kernel for nn_Decoder (greedy GRU decoder, 32 steps).

Model: B=64, E=512, H=1024, V=32000, STEPS=32, greedy argmax token feedback.

Distribution over 8 NeuronCores:
  - fc weight [V, H] sharded over V: 4000 rows/core (padded to 4096).
  - GRU gate weights sharded over the hidden dim: each core computes the
    r/z/n slices for its 128 h-columns.
  - The hidden state is exchanged each step with an AllGather of the
    [128, 64] transposed h-shard; argmax/softmax stats are exchanged with a
    second small AllGather. Greedy feedback (argmax over the full vocab) is
    reconstructed on every core identically.
  - Matmuls run in float32r (~4x fp32 rate); validated token-exact vs the
    fp32 reference for these inputs.

The kernel outputs raw logits plus per-core local (max, sumexp) stats;
the host applies logp = logits - (m_glob + log(sumexp_glob)) and assembles
the full outputs.
"""

import numpy as np

P = 128
B = 64
E = 512
H = 1024
V = 32000
VS = 4000  # vocab shard per core
VP = 4096  # padded vocab shard
G = 384  # 3 * 128 gate-slice width per core
NC = 8
STEPS = 32
SOS = 1
KE = E // P  # 4
KH = H // P  # 8
NCHUNK = VP // 512  # 8
NEG_BIG = -1.0e30
IDX_BIG = 1.0e9

_PROGRAM_CACHE = {}


def _build_program():
    import concourse.bacc as bacc
    import concourse.mybir as mybir
    import concourse.tile as tile
    from concourse.masks import make_identity

    f32 = mybir.dt.float32
    f32r = mybir.dt.float32r
    i32 = mybir.dt.int32
    u32 = mybir.dt.uint32
    u8 = mybir.dt.uint8
    AF = mybir.ActivationFunctionType
    OP = mybir.AluOpType

    nc = bacc.Bacc("TRN2", target_bir_lowering=False, debug=False, num_devices=NC)

    # ---- I/O ----
    emb_d = nc.dram_tensor("emb", [V, E], f32, kind="ExternalInput")
    wihT_d = nc.dram_tensor("wihT", [E, G], f32, kind="ExternalInput")
    whhT_d = nc.dram_tensor("whhT", [H, G], f32, kind="ExternalInput")
    fcwT_d = nc.dram_tensor("fcwT", [H, VP], f32, kind="ExternalInput")
    bih_d = nc.dram_tensor("bih", [B, G], f32, kind="ExternalInput")
    bhh_d = nc.dram_tensor("bhh", [B, G], f32, kind="ExternalInput")
    fcb_d = nc.dram_tensor("fcb", [B, VP], f32, kind="ExternalInput")
    voc_d = nc.dram_tensor("voc", [B, 1], f32, kind="ExternalInput")
    cbase_d = nc.dram_tensor("cbase", [B, NCHUNK], f32, kind="ExternalInput")
    x0T_d = nc.dram_tensor("x0T", [E, B], f32, kind="ExternalInput")
    h0T_d = nc.dram_tensor("h0T", [H, B], f32, kind="ExternalInput")
    h0s_d = nc.dram_tensor("h0s", [B, P], f32, kind="ExternalInput")

    logits_out = nc.dram_tensor("logits_out", [STEPS, B, VS], f32, kind="ExternalOutput")
    mstats_out = nc.dram_tensor("mstats_out", [STEPS, B, 2], f32, kind="ExternalOutput")
    hT_out = nc.dram_tensor("hT_out", [B, P], f32, kind="ExternalOutput")

    # collective bounce buffers
    h_ag_in = nc.dram_tensor("h_ag_in", [P, B], f32)
    h_ag_outs = [
        nc.dram_tensor(f"h_ag_out_{t}", [H, B], f32, addr_space="Shared")
        for t in range(STEPS)
    ]
    st_ag_in = nc.dram_tensor("st_ag_in", [B, 4], f32)
    st_ag_outs = [
        nc.dram_tensor(f"st_ag_out_{t}", [NC * B, 4], f32, addr_space="Shared")
        for t in range(STEPS - 1)
    ]
    groups = [list(range(NC))]

    with tile.TileContext(nc) as tc:
        with (
            tc.tile_pool(name="wconst", bufs=1) as wconst,
            tc.tile_pool(name="work", bufs=2) as work,
            tc.tile_pool(name="logits", bufs=2) as lpool,
            tc.tile_pool(name="small", bufs=1) as small,
            tc.tile_pool(name="fcps", bufs=2, space="PSUM") as fcps,
            tc.tile_pool(name="gps", bufs=1, space="PSUM") as gps,
            tc.tile_pool(name="tps", bufs=1, space="PSUM") as tps,
        ):
            # ---- constants / weights in SBUF ----
            ident = wconst.tile([P, P], f32)
            make_identity(nc, ident[:])

            fcw_sb = wconst.tile([P, KH, VP], f32)
            for k in range(KH):
                nc.sync.dma_start(fcw_sb[:, k, :], fcwT_d.ap()[k * P : (k + 1) * P, :])
            whh_sb = wconst.tile([P, KH, G], f32)
            for k in range(KH):
                nc.sync.dma_start(whh_sb[:, k, :], whhT_d.ap()[k * P : (k + 1) * P, :])
            wih_sb = wconst.tile([P, KE, G], f32)
            for k in range(KE):
                nc.sync.dma_start(wih_sb[:, k, :], wihT_d.ap()[k * P : (k + 1) * P, :])

            bih_sb = wconst.tile([B, G], f32)
            nc.sync.dma_start(bih_sb[:], bih_d.ap())
            bhh_sb = wconst.tile([B, G], f32)
            nc.sync.dma_start(bhh_sb[:], bhh_d.ap())
            fcb_sb = wconst.tile([P, VP], f32)
            nc.sync.dma_start(fcb_sb[0:B, :], fcb_d.ap())
            nc.sync.dma_start(fcb_sb[B:P, :], fcb_d.ap())
            voc_sb = wconst.tile([B, 1], f32)
            nc.sync.dma_start(voc_sb[:], voc_d.ap())
            cbase_sb = wconst.tile([B, NCHUNK], f32)
            nc.sync.dma_start(cbase_sb[:], cbase_d.ap())
            bigt = wconst.tile([B, NCHUNK], f32)
            nc.vector.memset(bigt[:], IDX_BIG)

            # ---- initial state ----
            hT_cur = work.tile([P, KH, B], f32, tag="hT")
            nc.sync.dma_start(hT_cur[:], h0T_d.ap().rearrange("(k p) b -> p k b", p=P))
            xT_cur = work.tile([P, KE, B], f32, tag="xT")
            nc.sync.dma_start(xT_cur[:], x0T_d.ap().rearrange("(k p) b -> p k b", p=P))
            h_old = work.tile([B, P], f32, tag="hsh")
            nc.sync.dma_start(h_old[:], h0s_d.ap())

            for t in range(STEPS):
                # ---- GRU gate matmuls (float32r) ----
                gh_ps = gps.tile([B, G], f32, tag="gh")
                for k in range(KH):
                    nc.tensor.matmul(
                        gh_ps[:], lhsT=hT_cur[:, k, :], rhs=whh_sb[:, k, :],
                        start=(k == 0), stop=(k == KH - 1),
                    )
                gi_ps = gps.tile([B, G], f32, tag="gi")
                for k in range(KE):
                    nc.tensor.matmul(
                        gi_ps[:], lhsT=xT_cur[:, k, :], rhs=wih_sb[:, k, :],
                        start=(k == 0), stop=(k == KE - 1),
                    )

                # ---- GRU elementwise (batch-partition layout) ----
                gh_sb = small.tile([B, G], f32, tag="ghs")
                nc.vector.tensor_tensor(out=gh_sb[:], in0=gh_ps[:], in1=bhh_sb[:], op=OP.add)
                gi_sb = small.tile([B, G], f32, tag="gis")
                nc.vector.tensor_tensor(out=gi_sb[:], in0=gi_ps[:], in1=bih_sb[:], op=OP.add)
                s_rz = small.tile([B, 256], f32, tag="srz")
                nc.vector.tensor_tensor(
                    out=s_rz[:], in0=gi_sb[:, 0:256], in1=gh_sb[:, 0:256], op=OP.add
                )
                t_rz = small.tile([B, 256], f32, tag="trz")
                nc.scalar.activation(t_rz[:], s_rz[:], AF.Tanh, scale=0.5)

                hn = gh_sb[:, 256:384]
                inb = gi_sb[:, 256:384]
                u = small.tile([B, P], f32, tag="u")
                nc.vector.tensor_tensor(out=u[:], in0=t_rz[:, 0:P], in1=hn[:], op=OP.mult)
                nc.vector.tensor_tensor(out=u[:], in0=u[:], in1=hn[:], op=OP.add)
                narg = small.tile([B, P], f32, tag="narg")
                nc.vector.tensor_scalar(
                    out=narg[:], in0=u[:], scalar1=0.5, scalar2=None, op0=OP.mult
                )
                nc.vector.tensor_tensor(out=narg[:], in0=narg[:], in1=inb[:], op=OP.add)
                n_t = small.tile([B, P], f32, tag="nt")
                nc.scalar.activation(n_t[:], narg[:], AF.Tanh)

                d = small.tile([B, P], f32, tag="d")
                nc.vector.tensor_tensor(out=d[:], in0=h_old[:], in1=n_t[:], op=OP.subtract)
                e = small.tile([B, P], f32, tag="e")
                nc.vector.tensor_tensor(out=e[:], in0=t_rz[:, P:256], in1=d[:], op=OP.mult)
                nc.vector.tensor_tensor(out=e[:], in0=e[:], in1=d[:], op=OP.add)
                h_new = work.tile([B, P], f32, tag="hsh")
                nc.vector.tensor_scalar(
                    out=h_new[:], in0=e[:], scalar1=0.5, scalar2=None, op0=OP.mult
                )
                nc.vector.tensor_tensor(out=h_new[:], in0=h_new[:], in1=n_t[:], op=OP.add)

                # ---- transpose h shard, AllGather h^T ----
                h_tp = tps.tile([P, B], f32, tag="htp")
                nc.tensor.transpose(h_tp[:], h_new[:], ident[0:B, 0:B])
                hTsh = small.tile([P, B], f32, tag="hTsh")
                nc.vector.tensor_copy(hTsh[:], h_tp[:])
                nc.sync.dma_start(h_ag_in.ap(), hTsh[:])
                nc.gpsimd.collective_compute(
                    "AllGather", mybir.AluOpType.bypass, replica_groups=groups,
                    ins=[h_ag_in.ap().opt()], outs=[h_ag_outs[t].ap().opt()],
                )
                hT_next = work.tile([P, KH, B], f32, tag="hT")
                nc.sync.dma_start(
                    hT_next[:], h_ag_outs[t].ap().rearrange("(k p) b -> p k b", p=P)
                )

                # ---- fc matmul over 8 chunks of 512, with fused stats ----
                cmax = small.tile([B, NCHUNK, 8], f32, tag="cmax")
                cidx = small.tile([B, NCHUNK, 8], u32, tag="cidx")
                mnegs = small.tile([B, NCHUNK], f32, tag="mnegs")
                se8 = small.tile([B, NCHUNK], f32, tag="se8")
                cmaxB = small.tile([P, 4, 8], f32, tag="cmaxB")
                cidxB = small.tile([P, 4, 8], u32, tag="cidxB")
                mnegsB = small.tile([P, 4], f32, tag="mnegsB")
                se8B = small.tile([P, 4], f32, tag="se8B")
                # chunk pair (2*cp, 2*cp+1) computed concurrently via PE column
                # tiling: even chunk -> array cols 0-63 (psum partitions 0-63),
                # odd chunk -> cols 64-127 (psum partitions 64-127).
                for cp in range(4):
                    c0, c1 = 2 * cp, 2 * cp + 1
                    psA = fcps.tile([P, 512], f32, tag="fcA")
                    psB = fcps.tile([P, 512], f32, tag="fcB")
                    for k in range(KH):
                        nc.tensor.matmul(
                            psA[0:B, :], lhsT=hT_next[:, k, :],
                            rhs=fcw_sb[:, k, c0 * 512 : (c0 + 1) * 512],
                            start=(k == 0), stop=(k == KH - 1),
                            tile_position=(0, 0),
                        )
                        nc.tensor.matmul(
                            psB[B:P, :], lhsT=hT_next[:, k, :],
                            rhs=fcw_sb[:, k, c1 * 512 : (c1 + 1) * 512],
                            start=(k == 0), stop=(k == KH - 1),
                            tile_position=(0, B),
                        )
                    lchunk = lpool.tile([B, 512], f32, tag="lchunk")
                    sl = lchunk[:]
                    nc.vector.tensor_tensor(
                        out=sl, in0=psA[0:B, :],
                        in1=fcb_sb[0:B, c0 * 512 : (c0 + 1) * 512],
                        op=OP.add,
                    )
                    nc.sync.dma_start(
                        logits_out.ap()[t, :, c0 * 512 : (c0 + 1) * 512], sl
                    )
                    nc.vector.max(out=cmax[:, cp, :], in_=sl)
                    nc.vector.max_index(out=cidx[:, cp, :], in_max=cmax[:, cp, :], in_values=sl)
                    nc.vector.tensor_scalar(
                        out=mnegs[:, cp : cp + 1], in0=cmax[:, cp, 0:1],
                        scalar1=-1.0, scalar2=None, op0=OP.mult,
                    )
                    esc = small.tile([B, 512], f32, tag="esc")
                    nc.scalar.activation(
                        esc[:], sl, AF.Exp, bias=mnegs[:, cp : cp + 1],
                        accum_out=se8[:, cp : cp + 1],
                    )
                    # odd chunk on partitions 64-127
                    lchunkB = lpool.tile([P, 512], f32, tag="lchunkB")
                    slB = lchunkB[B:P, :]
                    nc.vector.tensor_tensor(
                        out=slB, in0=psB[B:P, :],
                        in1=fcb_sb[B:P, c1 * 512 : (c1 + 1) * 512],
                        op=OP.add,
                    )
                    lo, hi = c1 * 512, min((c1 + 1) * 512, VS)
                    if lo < VS:
                        nc.sync.dma_start(
                            logits_out.ap()[t, :, lo:hi], slB[:, 0 : hi - lo]
                        )
                    nc.vector.max(out=cmaxB[B:P, cp, :], in_=slB)
                    nc.vector.max_index(
                        out=cidxB[B:P, cp, :], in_max=cmaxB[B:P, cp, :], in_values=slB
                    )
                    nc.vector.tensor_scalar(
                        out=mnegsB[B:P, cp : cp + 1], in0=cmaxB[B:P, cp, 0:1],
                        scalar1=-1.0, scalar2=None, op0=OP.mult,
                    )
                    escB = small.tile([P, 512], f32, tag="escB")
                    nc.scalar.activation(
                        escB[B:P, :], slB, AF.Exp, bias=mnegsB[B:P, cp : cp + 1],
                        accum_out=se8B[B:P, cp : cp + 1],
                    )
                # move odd-chunk stats down to partitions 0-63, slots 4-7
                nc.sync.dma_start(cmax[:, 4:8, :], cmaxB[B:P, :, :])
                nc.sync.dma_start(cidx[:, 4:8, :], cidxB[B:P, :, :])
                nc.sync.dma_start(se8[:, 4:8], se8B[B:P, :])

                # ---- local stats: m_l, idx_l, se_l ----
                m8 = small.tile([B, 8], f32, tag="m8")
                nc.vector.max(out=m8[:], in_=cmax[:, :, 0])
                m_l = m8[:, 0:1]
                mlneg = small.tile([B, 1], f32, tag="mlneg")
                nc.vector.tensor_scalar(
                    out=mlneg[:], in0=m_l, scalar1=-1.0, scalar2=None, op0=OP.mult
                )
                # exp(m_c - m_l) * se_c summed
                e8 = small.tile([B, NCHUNK], f32, tag="e8")
                nc.scalar.activation(e8[:], cmax[:, :, 0], AF.Exp, bias=mlneg[:])
                nc.vector.tensor_tensor(out=e8[:], in0=e8[:], in1=se8[:], op=OP.mult)
                se_l = small.tile([B, 1], f32, tag="sel")
                nc.vector.tensor_reduce(
                    out=se_l[:], in_=e8[:], axis=mybir.AxisListType.X, op=OP.add
                )
                # local argmax index (global vocab id)
                cidxf = small.tile([B, NCHUNK], f32, tag="cidxf")
                nc.vector.tensor_copy(cidxf[:], cidx[:, :, 0])
                nc.vector.tensor_tensor(out=cidxf[:], in0=cidxf[:], in1=cbase_sb[:], op=OP.add)
                nc.vector.tensor_scalar(
                    out=cidxf[:], in0=cidxf[:], scalar1=voc_sb[:, 0:1], scalar2=None,
                    op0=OP.add,
                )
                eq = small.tile([B, NCHUNK], u8, tag="eq")
                nc.vector.tensor_scalar(
                    out=eq[:], in0=cmax[:, :, 0], scalar1=m_l, scalar2=None, op0=OP.is_equal
                )
                msk = small.tile([B, NCHUNK], f32, tag="msk")
                nc.vector.select(msk[:], eq[:], cidxf[:], bigt[:])
                idx_l = small.tile([B, 1], f32, tag="idxl")
                nc.vector.tensor_reduce(
                    out=idx_l[:], in_=msk[:], axis=mybir.AxisListType.X, op=OP.min
                )

                # ---- outputs: local stats ----
                mst = small.tile([B, 2], f32, tag="mst")
                nc.vector.tensor_copy(mst[:, 0:1], m_l)
                nc.vector.tensor_copy(mst[:, 1:2], se_l[:])
                nc.sync.dma_start(mstats_out.ap()[t], mst[:])

                if t == STEPS - 1:
                    break

                # ---- stats AllGather -> global argmax -> next token ----
                payload = small.tile([B, 4], f32, tag="payload")
                nc.vector.tensor_copy(payload[:, 0:1], m_l)
                nc.vector.tensor_copy(payload[:, 1:2], idx_l[:])
                nc.vector.tensor_copy(payload[:, 2:3], se_l[:])
                nc.vector.tensor_copy(payload[:, 3:4], se_l[:])
                nc.sync.dma_start(st_ag_in.ap(), payload[:])
                nc.gpsimd.collective_compute(
                    "AllGather", mybir.AluOpType.bypass, replica_groups=groups,
                    ins=[st_ag_in.ap().opt()], outs=[st_ag_outs[t].ap().opt()],
                )
                statsall = small.tile([B, NC, 4], f32, tag="statsall")
                nc.sync.dma_start(
                    statsall[:], st_ag_outs[t].ap().rearrange("(c b) s -> b c s", b=B)
                )
                mg8 = small.tile([B, 8], f32, tag="mg8")
                nc.vector.max(out=mg8[:], in_=statsall[:, :, 0])
                m_g = mg8[:, 0:1]
                eqg = small.tile([B, NC], u8, tag="eqg")
                nc.vector.tensor_scalar(
                    out=eqg[:], in0=statsall[:, :, 0], scalar1=m_g, scalar2=None,
                    op0=OP.is_equal,
                )
                mskg = small.tile([B, NC], f32, tag="mskg")
                nc.vector.select(mskg[:], eqg[:], statsall[:, :, 1], bigt[:])
                tok_f = small.tile([B, 1], f32, tag="tokf")
                nc.vector.tensor_reduce(
                    out=tok_f[:], in_=mskg[:], axis=mybir.AxisListType.X, op=OP.min
                )
                tok_i = small.tile([B, 1], i32, tag="toki")
                nc.vector.tensor_copy(tok_i[:], tok_f[:])

                # ---- gather next x, transpose to x^T ----
                import concourse.bass as bass

                x_sb = small.tile([B, E], f32, tag="xsb")
                nc.gpsimd.indirect_dma_start(
                    out=x_sb[:], out_offset=None, in_=emb_d.ap(),
                    in_offset=bass.IndirectOffsetOnAxis(ap=tok_i[:, 0:1], axis=0),
                )
                xT_next = work.tile([P, KE, B], f32, tag="xT")
                for k in range(KE):
                    x_tp = tps.tile([P, B], f32, tag="xtp")
                    nc.tensor.transpose(
                        x_tp[:], x_sb[:, k * P : (k + 1) * P], ident[0:B, 0:B]
                    )
                    nc.vector.tensor_copy(xT_next[:, k, :], x_tp[:])

                hT_cur = hT_next
                xT_cur = xT_next
                h_old = h_new

            # final hidden state shard (exact fp32)
            nc.sync.dma_start(hT_out.ap(), h_new[:])

    nc.compile()
    return nc


def _prep_inputs(inputs):
    """Build per-core input maps from the full problem inputs."""
    emb = np.ascontiguousarray(np.asarray(inputs["emb"], dtype=np.float32))
    W_ih = np.asarray(inputs["W_ih"], dtype=np.float32)
    W_hh = np.asarray(inputs["W_hh"], dtype=np.float32)
    b_ih = np.asarray(inputs["b_ih"], dtype=np.float32)
    b_hh = np.asarray(inputs["b_hh"], dtype=np.float32)
    fc_W = np.asarray(inputs["fc_W"], dtype=np.float32)
    fc_b = np.asarray(inputs["fc_b"], dtype=np.float32)
    h0 = np.asarray(inputs["encoder_hidden"], dtype=np.float32)[0]  # [B, H]

    x0 = emb[SOS]  # [E]
    x0T = np.ascontiguousarray(np.repeat(x0[:, None], B, axis=1))  # [E, B]
    h0T = np.ascontiguousarray(h0.T)  # [H, B]

    in_maps = []
    for c in range(NC):
        sl = np.r_[c * P : (c + 1) * P, H + c * P : H + (c + 1) * P,
                   2 * H + c * P : 2 * H + (c + 1) * P]
        wihT = np.ascontiguousarray(W_ih[sl, :].T)  # [E, G]
        whhT = np.ascontiguousarray(W_hh[sl, :].T)  # [H, G]
        bi = b_ih[sl]
        bh = b_hh[sl]
        bih = np.tile(bi[None, :], (B, 1))
        bhh = np.tile(bh[None, :], (B, 1))
        fcw = np.zeros((VP, H), np.float32)
        fcw[:VS] = fc_W[c * VS : (c + 1) * VS]
        fcwT = np.ascontiguousarray(fcw.T)  # [H, VP]
        fcb = np.full((B, VP), NEG_BIG, np.float32)
        fcb[:, :VS] = fc_b[c * VS : (c + 1) * VS]
        voc = np.full((B, 1), float(c * VS), np.float32)
        corder = np.array([0, 2, 4, 6, 1, 3, 5, 7], np.float32)
        cbase = np.tile(corder[None, :] * 512.0, (B, 1))
        h0s = np.ascontiguousarray(h0[:, c * P : (c + 1) * P])
        in_maps.append(
            {
                "emb": emb, "wihT": wihT, "whhT": whhT, "fcwT": fcwT,
                "bih": np.ascontiguousarray(bih), "bhh": np.ascontiguousarray(bhh),
                "fcb": fcb, "voc": voc,
                "cbase": cbase, "x0T": x0T, "h0T": h0T, "h0s": h0s,
            }
        )
    return in_maps


def _assemble(results):
    """Host-side: combine per-core outputs into the full (logp, hT) pair."""
    # logits: [STEPS, B, VS] per core -> [B, STEPS, V]
    logits = np.concatenate([r["logits_out"] for r in results], axis=2)  # [S, B, V]
    # local stats per core: [STEPS, B, 2] (m_l, se_l)
    m_l = np.stack([r["mstats_out"][:, :, 0] for r in results], axis=0)  # [NC, S, B]
    se_l = np.stack([r["mstats_out"][:, :, 1] for r in results], axis=0)
    m_g = m_l.max(axis=0)  # [S, B]
    se_g = (se_l * np.exp(m_l - m_g[None])).sum(axis=0)
    lse = m_g + np.log(se_g)  # [S, B]
    logp = logits - lse[:, :, None]
    logp = np.ascontiguousarray(logp.transpose(1, 0, 2))  # [B, S, V]
    hT = np.concatenate([r["hT_out"] for r in results], axis=1)  # [B, H]
    return logp.astype(np.float32), hT[None].astype(np.float32)


def kernel(**inputs):
    from concourse.bass_utils import run_bass_kernel_spmd

    if "prog" not in _PROGRAM_CACHE:
        _PROGRAM_CACHE["prog"] = _build_program()
    nc = _PROGRAM_CACHE["prog"]
    in_maps = _prep_inputs(inputs)
    res = run_bass_kernel_spmd(nc, in_maps, core_ids=list(range(NC)))
    return _assemble(res.results)


if __name__ == "__main__":
    d = np.load("/root/problem/ref_cache.npz")
    inp = {k: d[k] for k in ["target", "encoder_hidden", "emb", "W_ih", "W_hh",
                             "b_ih", "b_hh", "fc_W", "fc_b"]}
    out, hT = kernel(**inp)
    ref, ref_hT = d["out0"], d["out1"]
    err = np.abs(out - ref)
    denom = np.maximum(np.abs(ref), 1e-6)
    rel = np.linalg.norm(out - ref) / np.linalg.norm(ref)
    tok_k = np.argmax(out, axis=-1)
    tok_r = np.argmax(ref, axis=-1)
    print(f"logp: max_abs={err.max():.3e} rel_fro={rel:.3e} tok_mismatch={(tok_k != tok_r).sum()}")
    relh = np.linalg.norm(hT - ref_hT) / np.linalg.norm(ref_hT)
    print(f"hT: rel_fro={relh:.3e} max_abs={np.abs(hT - ref_hT).max():.3e}")
